# revision 6
# baseline (speedup 1.0000x reference)
"""Trainium2 Bass kernel for nn_BlockGC (gnn_message_passing) — v2.

Sharding: data-parallel over batch N=16 across 8 NeuronCores (2/core).
Exact BatchNorm batch stats via one tiny AllReduce.

Key layout/engine decisions (vs v1 baseline):
 - Host pre-packs x into the two SBUF layouts the kernel needs, so the
   device does 3 big contiguous DMAs (xT, x_nat, wf) instead of ~260
   small strided ones.
 - Main GEMM per (head h, sample n): PSUM[t, (w,o')] accumulated over 4
   contraction chunks of (c', v)=512; channels stay fused per head
   (Wf = wg*BnA fused on host).
 - BN stats on PE: Σval via ones-column matmuls over the bf16 value
   tiles; residual Σval via xsum; residual Σval² via Gram diagonal
   (G = RᵀR accumulated on PE, diag extracted with an identity mask).
 - Combine phase runs in a (tm, o')-partition layout produced by DVE
   32x32 block transposes, so BN coefficients are per-partition scalars
   and the whole affine+relu is 2 DVE passes + 1 Act pass per head.
 - Output DMA writes (ts, w)=3200B contiguous runs per partition.
"""

import numpy as np

N, C, T, V = 16, 128, 128, 25
K, H, OC = 3, 8, 256
EPS_BN = 1e-5
EPS_NORM = 1e-4
NCORES = 8
NS = N // NCORES          # samples per core
CH = C // H               # 16
OCH = OC // H             # 32
VP = 32                   # padded V
M_FREE = OCH * V          # 800 = (o', w) free block per head
NTOT = N * T * V          # batchnorm sample count per channel

_CACHED = {}


def _host_prep(inputs):
    import ml_dtypes
    bf16 = ml_dtypes.bfloat16

    x = np.asarray(inputs["x"], np.float32)
    hop = np.asarray(inputs["hop"])
    emb = np.asarray(inputs["emb_table"], np.float32)
    A = np.asarray(inputs["A"], np.float32)
    w_block = np.asarray(inputs["w_block"], np.float32)
    res_w = np.asarray(inputs["res_w"], np.float32)

    B = emb[:, :, hop]                                  # [K,H,V,V]

    def coln(w):
        return np.sqrt((w * w).sum(axis=-2, keepdims=True)) + EPS_NORM

    BnA = B / coln(B) + A / coln(A)                     # [K,H,V,V]

    wg = w_block.reshape(K, H, OCH, CH)                 # [K,H,o',c']
    # fused weight, free dim in (w, o') order to match fin layout
    Wf = np.einsum("khoc,khvw->hcvwo", wg, BnA)         # [H,CH,V,V,OCH]
    Wf_p = np.zeros((H, CH, VP, V, OCH), np.float32)
    Wf_p[:, :, :V] = Wf
    # contraction rows: c' = 4r + a ; partition p = 32a + v
    Wf_dev = Wf_p.reshape(H, 4, 4, VP, M_FREE)          # [H,r,a,v,(w o)]
    Wf_dev = np.ascontiguousarray(
        Wf_dev.reshape(H, 4, 128, M_FREE).astype(bf16))
    # w-summed weights for the Σval shortcut: Σ_w Wf[.., w, o]
    Wfsum = Wf_p.sum(axis=3)                             # [H,CH,VP,OCH]
    Wfsum = np.ascontiguousarray(
        Wfsum.reshape(H, 4, 128, OCH).astype(bf16))

    xb = x.astype(bf16)                                  # [N,C,T,V]
    xp = np.zeros((N, C, T, VP), bf16)
    xp[..., :V] = xb

    # x_nat: [core][c, n, t, v32] contiguous
    x_nat = np.ascontiguousarray(
        xp.reshape(NCORES, NS, C, T, VP).transpose(0, 2, 1, 3, 4))

    # xT: [core][p=(a,v32), h, r, n, t] contiguous
    #   c = 16h + 4r + a
    x6 = xp.reshape(NCORES, NS, H, 4, 4, T, VP)          # [co,n,h,r,a,t,v]
    xT = np.ascontiguousarray(x6.transpose(0, 4, 6, 2, 3, 1, 5)) \
        .reshape(NCORES, 128, H, 4, NS, T)               # [(a v),h,r,n,t]

    rwT = np.ascontiguousarray(res_w.T.astype(bf16))     # [C, OC]

    gb = np.ascontiguousarray(np.concatenate([
        np.asarray(inputs["bn_gamma"], np.float32),
        np.asarray(inputs["bn_beta"], np.float32),
        np.asarray(inputs["res_bn_gamma"], np.float32),
        np.asarray(inputs["res_bn_beta"], np.float32),
    ])[None, :])                                         # [1, 4*256]

    sel = np.zeros((32, 128), np.float32)                # selector for coef
    for p in range(128):
        sel[p % 32, p] = 1.0
    ident = np.ascontiguousarray(np.eye(128, dtype=np.float32))

    # one-hot column blocks: eyerep[:, 8h+j] = (j == h), for stat matmuls
    eyerep = np.zeros((128, H * H), bf16)
    for h in range(H):
        eyerep[:, H * h + h] = 1.0

    # per-partition-o' gamma / beta12 for the [32, *] coef math
    g1 = np.asarray(inputs["bn_gamma"], np.float32).reshape(H, OCH)
    g2 = np.asarray(inputs["res_bn_gamma"], np.float32).reshape(H, OCH)
    b1 = np.asarray(inputs["bn_beta"], np.float32).reshape(H, OCH)
    b2 = np.asarray(inputs["res_bn_beta"], np.float32).reshape(H, OCH)
    gb32 = np.ascontiguousarray(
        np.stack([g1, g2], axis=0).transpose(2, 0, 1))   # [32, 2, 8]
    bet12 = np.ascontiguousarray((b1 + b2).T)            # [32, 8]

    return (xT, x_nat, Wf_dev, Wfsum, rwT, gb, sel, ident, eyerep, gb32,
            bet12)


# ---------------------------------------------------------------------------
# Post-pass: this walrus build only accepts ONE sync wait / update command per
# instruction.  Split excess waits onto NOPs inserted before the instruction
# (same engine), excess updates onto NOPs after it.
# ---------------------------------------------------------------------------
def _split_excess_sync(nc, max_waits=1, max_updates=1):
    import bass_rust
    import concourse.mybir as mybir

    eng_map = None

    def make_nop(engine):
        nonlocal eng_map
        if eng_map is None:
            eng_map = {
                mybir.EngineType.SP: nc.sync,
                mybir.EngineType.DVE: nc.vector,
                mybir.EngineType.Activation: nc.scalar,
                mybir.EngineType.PE: nc.tensor,
                mybir.EngineType.Pool: nc.gpsimd,
            }
        bi = eng_map[engine].nop()
        inst = bi.ins
        f = nc.m.functions[0]
        for bb in f.blocks:
            names = [i.name for i in bb.instructions]
            if inst.name in names:
                lst = list(bb.instructions)
                lst.pop(names.index(inst.name))
                bb.instructions = lst
                break
        return inst

    f = nc.m.functions[0]
    for bb in f.blocks:
        insts = list(bb.instructions)
        out = []
        changed = False
        for inst in insts:
            si = inst.sync_info
            waits = list(si.on_wait) if si and si.on_wait else []
            ups = list(si.on_update) if si and si.on_update else []
            if len(waits) > max_waits:
                excess = waits[:-max_waits]
                keep = waits[-max_waits:]
                for i in range(0, len(excess), max_waits):
                    nop = make_nop(inst.engine)
                    nop.sync_info = bass_rust.SyncInfo(
                        on_wait=excess[i:i + max_waits], on_update=[])
                    out.append(nop)
                inst.sync_info = bass_rust.SyncInfo(on_wait=keep, on_update=ups)
                changed = True
            out.append(inst)
            if len(ups) > max_updates:
                keep_u = ups[:max_updates]
                excess_u = ups[max_updates:]
                si2 = inst.sync_info
                inst.sync_info = bass_rust.SyncInfo(
                    on_wait=list(si2.on_wait or []), on_update=keep_u)
                for i in range(0, len(excess_u), max_updates):
                    nop = make_nop(inst.engine)
                    nop.sync_info = bass_rust.SyncInfo(
                        on_wait=[], on_update=excess_u[i:i + max_updates])
                    out.append(nop)
                changed = True
        if changed:
            bb.instructions = out


def _build_bass():
    import concourse.bass as bass
    import concourse.mybir as mybir
    import concourse.tile as tile

    f32 = mybir.dt.float32
    bf16 = mybir.dt.bfloat16
    Alu = mybir.AluOpType
    Act = mybir.ActivationFunctionType

    nc = bass.Bass(num_devices=NCORES)

    xTp = nc.declare_dram_parameter("xT", [128, H, 4, NS, T], bf16,
                                    isOutput=False)
    xnp = nc.declare_dram_parameter("xnat", [128, NS, T, VP], bf16,
                                    isOutput=False)
    wfp = nc.declare_dram_parameter("wf", [H, 4, 128, M_FREE], bf16,
                                    isOutput=False)
    wfsp = nc.declare_dram_parameter("wfs", [H, 4, 128, OCH], bf16,
                                     isOutput=False)
    rwp = nc.declare_dram_parameter("rwT", [C, OC], bf16, isOutput=False)
    selp = nc.declare_dram_parameter("sel", [32, 128], f32, isOutput=False)
    idp = nc.declare_dram_parameter("ident", [128, 128], f32, isOutput=False)
    eyep = nc.declare_dram_parameter("eyerep", [128, H * H], bf16,
                                     isOutput=False)
    g32p = nc.declare_dram_parameter("gb32", [32, 2, H], f32, isOutput=False)
    b12p = nc.declare_dram_parameter("bet12", [32, H], f32, isOutput=False)
    # output split into one param per (head-pair, tm) so the store DMAs
    # don't WAW-chain on a single DRAM tensor; host reassembles.
    outs = [
        nc.declare_dram_parameter(f"out_{hp}_{tm}", [2, NS, 32, 32, V], f32,
                                  isOutput=True)
        for hp in range(H // 2) for tm in range(4)
    ]

    cc_in = nc.dram_tensor("cc_in", [1, 4 * OC], f32)
    cc_out = nc.dram_tensor("cc_out", [1, 4 * OC], f32, addr_space="Shared")

    with tile.TileContext(nc) as tc:
        with (
            tc.tile_pool(name="vals", bufs=1) as p_vals,
            tc.tile_pool(name="small", bufs=1) as p_small,
        ):
            # ---- small constants ----
            rw_sb = p_small.tile([128, OC], bf16, tag="rw")
            nc.sync.dma_start(rw_sb[:], rwp[:])
            sel_sb = p_small.tile([32, 128], f32, tag="sel")
            nc.sync.dma_start(sel_sb[:], selp[:])
            id_sb = p_small.tile([128, 128], f32, tag="ident")
            nc.sync.dma_start(id_sb[:], idp[:])
            ones_b = p_small.tile([128, 1], bf16, tag="onesb")
            nc.vector.memset(ones_b[:], 1.0)
            ones_f = p_small.tile([128, 1], f32, tag="onesf")
            nc.vector.memset(ones_f[:], 1.0)
            eye_sb = p_small.tile([128, H * H], bf16, tag="eyerep")
            nc.sync.dma_start(eye_sb[:], eyep[:])
            g32_sb = p_small.tile([32, 2, H], f32, tag="gb32")
            nc.sync.dma_start(g32_sb[:], g32p[:])
            b12_sb = p_small.tile([32, H], f32, tag="bet12")
            nc.sync.dma_start(b12_sb[:], b12p[:])
            eps_ap = p_small.tile([32, 1], f32, tag="eps")
            nc.vector.memset(eps_ap[:], EPS_BN)

            # ---- value tensors (outlive GEMM-phase pools) ----
            # fin: [p=t(tm,ts), n, w, h, o']
            fin = p_vals.tile([128, NS, V, H, OCH], bf16, tag="fin")
            # res_fin: [p=t, n, w, (h o')=oc]
            res_fin = p_vals.tile([128, NS, V, OC], bf16, tag="resfin")
            # val stat rows [mval 256 | rval 256] and residual sq row
            valrow = p_vals.tile([1, 512], f32, tag="valrow")
            resrow = p_vals.tile([1, 256], f32, tag="resrow")

            with (
                tc.tile_pool(name="xbig", bufs=1) as p_x,
                tc.tile_pool(name="scr", bufs=2) as p_scr,
            ):
                xT_sb = p_x.tile([128, H, 4, NS, T], bf16, tag="xT")
                nc.sync.dma_start(xT_sb[:], xTp[:])
                wf_sb = p_x.tile([128, H, 4, M_FREE], bf16, tag="wf")
                for h in range(H):
                    nc.sync.dma_start(wf_sb[:, h],
                                      wfp[h].rearrange("r p m -> p r m"))
                x_nat = p_x.tile([128, NS, T, VP], bf16, tag="xnat")
                nc.sync.dma_start(x_nat[:], xnp[:])
                wfs_sb = p_x.tile([128, H, 4, OCH], bf16, tag="wfs")
                nc.sync.dma_start(wfs_sb[:],
                                  wfsp[:].rearrange("h r p m -> p h r m"))

                # Σ_{n,t} x in both layouts (for the Σval shortcuts)
                xsm = p_x.tile([128, H, 4], f32, tag="xsm")
                nc.vector.reduce_sum(
                    xsm[:], xT_sb[:].rearrange("p h r n t -> p h r (n t)"),
                    axis=mybir.AxisListType.X)
                xsm_b = p_x.tile([128, H, 4], bf16, tag="xsmb")
                nc.vector.tensor_copy(xsm_b[:], xsm[:])
                xsn = p_x.tile([128, 1], f32, tag="xsn")
                nc.vector.reduce_sum(
                    xsn[:], x_nat[:].rearrange("p n t v -> p (n t v)"),
                    axis=mybir.AxisListType.X)
                xsn_b = p_x.tile([128, 1], bf16, tag="xsnb")
                nc.vector.tensor_copy(xsn_b[:], xsn[:])

                # ---------------- main fused GEMMs + stats ----------------
                with (
                    tc.tile_pool(name="pm", bufs=2, space="PSUM") as p_pm,
                    tc.tile_pool(name="pst", bufs=1, space="PSUM") as p_pst,
                ):
                    # sq stat rows accumulate across ALL heads: row h of the
                    # [8, 800] psum receives head h's Σ_t via a one-hot
                    # column stationary (eyerep[:, 8h:8h+8]).
                    ps_sq = p_pst.tile([8, M_FREE], f32, tag="pssq")
                    for h in range(H):
                        for n in range(NS):
                            pm = p_pm.tile([128, M_FREE], f32, tag="pmain")
                            for r in range(4):
                                st, sp = (r == 0), (r == 3)
                                nc.tensor.matmul(pm[:, 0:512],
                                                 xT_sb[:, h, r, n, :],
                                                 wf_sb[:, h, r, 0:512],
                                                 start=st, stop=sp)
                                nc.tensor.matmul(pm[:, 512:M_FREE],
                                                 xT_sb[:, h, r, n, :],
                                                 wf_sb[:, h, r, 512:M_FREE],
                                                 start=st, stop=sp)
                            # evict values (bf16), fin free = (w, o')
                            fv = fin[:, n, :, h, :]
                            nc.scalar.activation(
                                fv, pm[:, 0:M_FREE].rearrange(
                                    "p (w o) -> p w o", w=V, o=OCH),
                                Act.Copy)
                            # squares (bf16, 2x DVE)
                            scr = p_scr.tile([128, V, OCH], bf16, tag="sq")
                            nc.vector.tensor_mul(scr[:], fv, fv)
                            # sq stat matmuls: Σ_t into row h of ps_sq
                            st = (h == 0 and n == 0)
                            sp = (h == H - 1 and n == NS - 1)
                            eh = eye_sb[:, H * h:H * (h + 1)]
                            nc.tensor.matmul(ps_sq[:, 0:512], eh,
                                             scr[:, 0:16, :],
                                             start=st, stop=sp)
                            nc.tensor.matmul(ps_sq[:, 512:M_FREE], eh,
                                             scr[:, 16:V, :],
                                             start=st, stop=sp)

                    # Σval via the w-summed weights: val[h,o'] =
                    #   Σ_r xsumᵀ(h,r) · Wfsum(h,r)
                    ps_mv = p_pst.tile([1, OC], f32, tag="psmv")
                    for h in range(H):
                        for r in range(4):
                            nc.tensor.matmul(ps_mv[:, 32 * h:32 * (h + 1)],
                                             xsm_b[:, h, r:r + 1],
                                             wfs_sb[:, h, r, :],
                                             start=(r == 0), stop=(r == 3))
                    nc.scalar.activation(valrow[:, 0:OC], ps_mv[:], Act.Copy)

                    # evict sq rows, reduce over w, ship to the AR input
                    mrow = p_scr.tile([8, M_FREE], f32, tag="mrow")
                    nc.scalar.activation(mrow[:], ps_sq[:], Act.Copy)
                    mred = p_scr.tile([8, OCH], f32, tag="mred")
                    nc.vector.reduce_sum(
                        mred[:],
                        mrow[:].rearrange("p (w o) -> p o w", w=V, o=OCH),
                        axis=mybir.AxisListType.X)
                    nc.sync.dma_start(cc_in[:, 512:768], mred[:])

                # -------- residual GEMMs + stats (interleaved) --------
                # Per (n, v): GEMM -> evict (Pool) -> Σval ones-matmul and
                # Gram accumulation ride in the PE stream right behind.
                with (
                    tc.tile_pool(name="pr", bufs=4, space="PSUM") as p_pr,
                    tc.tile_pool(name="prs", bufs=1, space="PSUM") as p_prs,
                ):
                    ps_rv = p_prs.tile([1, OC], f32, tag="psrv")
                    gps0 = p_prs.tile([128, 128], f32, tag="gram0")
                    gps1 = p_prs.tile([128, 128], f32, tag="gram1")
                    # Σ res values = xsum_natᵀ · rw
                    nc.tensor.matmul(ps_rv[:], xsn_b[:], rw_sb[:],
                                     start=True, stop=True)
                    nc.scalar.activation(valrow[:, OC:2 * OC], ps_rv[:],
                                         Act.Copy)
                    for n in range(NS):
                        for v in range(V):
                            pr = p_pr.tile([128, OC], f32, tag="pres")
                            nc.tensor.matmul(pr[:], x_nat[:, n, :, v],
                                             rw_sb[:], start=True, stop=True)
                            # GPSIMD can't read PSUM on HW; split evictions
                            # between Act and DVE
                            if v % 2 == 0:
                                nc.scalar.activation(res_fin[:, n, v, :],
                                                     pr[:], Act.Copy)
                            else:
                                nc.vector.tensor_copy(res_fin[:, n, v, :],
                                                      pr[:])
                            st = (n == 0 and v == 0)
                            sp = (n == NS - 1 and v == V - 1)
                            for c, gps in ((0, gps0), (1, gps1)):
                                sl = res_fin[:, n, v, 128 * c:128 * (c + 1)]
                                nc.tensor.matmul(gps[:], sl, sl,
                                                 start=st, stop=sp)
                    # Σ res² = diag(Gram)
                    for c, gps in ((0, gps0), (1, gps1)):
                        dsb = p_scr.tile([128, 128], f32, tag="diag")
                        nc.vector.tensor_mul(dsb[:], gps[:], id_sb[:])
                        ps_d = p_prs.tile([1, 128], f32, tag="psd")
                        nc.tensor.matmul(ps_d[:], ones_f[:], dsb[:],
                                         start=True, stop=True)
                        nc.scalar.activation(
                            resrow[:, 128 * c:128 * (c + 1)],
                            ps_d[:], Act.Copy)

            # ---------------- AllReduce ----------
            # AR payload: [mval 256 | rval 256 | msq 256 | rsq 256]
            # (mred -> [512:768] shipped above; spread DMAs across engines)
            nc.scalar.dma_start(cc_in[:, 0:512], valrow[:])
            nc.gpsimd.dma_start(cc_in[:, 768:1024], resrow[:])
            nc.gpsimd.collective_compute(
                "AllReduce", Alu.add,
                replica_groups=[list(range(NCORES))],
                ins=[cc_in[:]], outs=[cc_out[:]])
            # load reduced stats partition-spread: statg32[o', kind, h]
            statg = p_small.tile([32, 4, H], f32, tag="statg")
            nc.sync.dma_start(
                statg[:],
                cc_out[:].rearrange("one (k h o) -> (one o) k h",
                                    k=4, h=H, o=OCH))

            # ---------------- transposed (channel-major) values ---------
            with (
                tc.tile_pool(name="cm", bufs=1) as p_cm,
                tc.tile_pool(name="fo", bufs=2) as p_fo,
                tc.tile_pool(name="pc", bufs=1, space="PSUM") as p_pc,
            ):
                # main_cm/res_cm: [p=(tm,o'), n, w, h, ts]
                main_cm = p_cm.tile([128, NS, V, H, 32], bf16, tag="mcm")
                res_cm = p_cm.tile([128, NS, V, H, 32], bf16, tag="rcm")
                for h in range(H):
                    nc.vector.transpose(main_cm[:, :, :, h, :],
                                        fin[:, :, :, h, :])
                    nc.vector.transpose(
                        res_cm[:, :, :, h, :],
                        res_fin[:, :, :, :].rearrange(
                            "p n w (h o) -> p n w h o", h=H, o=OCH)[:, :, :, h, :])

                # ---------------- coefficients ----------------
                # statg [32, kind, h]: kinds (mval, rval, msq, rsq).
                # Everything on 32 partitions (one per o').
                coef32 = p_small.tile([32, 3, H], f32, tag="coef32")
                AB_v = coef32[:, 0:2, :]
                E_v = coef32[:, 2, :]
                mu = p_small.tile([32, 2, H], f32, tag="cmu")
                mu2 = p_small.tile([32, 2, H], f32, tag="cmu2")
                inv = 1.0 / float(NTOT)

                nc.vector.tensor_scalar_mul(mu[:], statg[:, 0:2, :], inv)
                nc.vector.tensor_mul(mu2[:], mu[:], mu[:])
                # var = sq/N - mu^2
                nc.vector.scalar_tensor_tensor(
                    AB_v, statg[:, 2:4, :], inv, mu2[:],
                    Alu.mult, Alu.subtract)
                # sd = sqrt(var + eps); A,B = gamma / sd
                nc.scalar.activation(AB_v, AB_v, Act.Sqrt, bias=eps_ap[:])
                nc.vector.reciprocal(AB_v, AB_v)
                nc.vector.tensor_mul(AB_v, AB_v, g32_sb[:])
                # E = (b1+b2) - A*mu_m - B*mu_r
                nc.vector.tensor_mul(mu2[:], AB_v, mu[:])
                nc.vector.tensor_sub(E_v, b12_sb[:], mu2[:, 0, :])
                nc.vector.tensor_sub(E_v, E_v, mu2[:, 1, :])

                # broadcast per-o' coef rows to all 128 partitions:
                # cb[p, (c,h)] = coef32[p % 32, c, h]
                cb_ps = p_pc.tile([128, 3 * H], f32, tag="cbps")
                nc.tensor.matmul(cb_ps[:], sel_sb[:],
                                 coef32[:].rearrange("o c h -> o (c h)"),
                                 start=True, stop=True)
                coef = p_small.tile([128, 3 * H], f32, tag="coef")
                nc.vector.tensor_copy(coef[:], cb_ps[:])

                # ---------------- combine + relu + out DMA ----------------
                # head-pair pipeline; each (head-pair, tm) block ships to
                # its own DRAM param with (ts,w)=3200B contiguous runs.
                # Engine assignment keeps SP/Pool even and gives Act the
                # tail DMAs (after its relus are done).
                dma_q = [nc.sync, nc.gpsimd, nc.sync, nc.gpsimd,
                         nc.sync, nc.gpsimd, nc.sync, nc.gpsimd,
                         nc.sync, nc.gpsimd, nc.scalar, nc.scalar,
                         nc.sync, nc.gpsimd, nc.scalar, nc.scalar]
                for h in range(H):
                    A_h = coef[:, h:h + 1]
                    B_h = coef[:, H + h:H + h + 1]
                    E_h = coef[:, 2 * H + h:2 * H + h + 1]
                    t1 = p_fo.tile([128, NS, V, 32], bf16, tag="t1")
                    nc.vector.tensor_scalar(t1[:], res_cm[:, :, :, h, :],
                                            B_h, E_h, Alu.mult, Alu.add)
                    t2b = p_fo.tile([128, NS, V, 32], bf16, tag="t2")
                    nc.vector.scalar_tensor_tensor(
                        t2b[:], main_cm[:, :, :, h, :], A_h, t1[:],
                        Alu.mult, Alu.add)
                    if h % 2 == 0:
                        fo2 = p_fo.tile([128, 2, NS, 32, V], f32, tag="fo")
                    nc.scalar.activation(
                        fo2[:, h % 2].rearrange("p n ts w -> p n w ts"),
                        t2b[:], Act.Relu)
                    if h % 2 == 1:
                        hp = h // 2
                        for tm in range(4):
                            dma_q[4 * hp + tm].dma_start(
                                outs[4 * hp + tm][:].rearrange(
                                    "g n o ts w -> o (g n) (ts w)"),
                                fo2[32 * tm:32 * (tm + 1)].rearrange(
                                    "p g n ts w -> p (g n) (ts w)"))

    _split_excess_sync(nc)
    return nc


def _make_runner(nc):
    """Build a cached PJRT executor (same lowering path run_bass_kernel_spmd
    uses under axon, but the jit closure is built once so warm calls skip
    re-trace/re-lower)."""
    import jax
    import jax.numpy as jnp
    from jax.sharding import Mesh, PartitionSpec
    from jax.experimental.shard_map import shard_map
    from concourse import bass2jax
    from concourse import mybir

    import jax.numpy as jnp

    bass2jax.install_neuronx_cc_hook()
    partition_name = (nc.partition_id_tensor.name
                      if nc.partition_id_tensor else None)
    # per-core (sharded) vs replicated inputs
    sharded_in = {"xT", "xnat"}
    in_names, out_names, out_avals, zero_outs = [], [], [], []
    for alloc in nc.m.functions[0].allocations:
        if not isinstance(alloc, mybir.MemoryLocationSet):
            continue
        name = alloc.memorylocations[0].name
        if alloc.kind == "ExternalInput":
            if name != partition_name:
                in_names.append(name)
        elif alloc.kind == "ExternalOutput":
            shape = tuple(alloc.tensor_shape)
            dtype = mybir.dt.np(alloc.dtype)
            out_names.append(name)
            out_avals.append(jax.core.ShapedArray(shape, dtype))
            zero_outs.append(np.zeros((NCORES * shape[0], *shape[1:]), dtype))
    n_params = len(in_names)
    all_names = list(in_names) + list(out_names)
    if partition_name is not None:
        all_names.append(partition_name)
    donate = tuple(range(n_params, n_params + len(out_names)))

    def _body(*args):
        operands = list(args)
        if partition_name is not None:
            operands.append(bass2jax.partition_id_tensor())
        return tuple(bass2jax._bass_exec_p.bind(
            *operands,
            out_avals=tuple(out_avals),
            in_names=tuple(all_names),
            out_names=tuple(out_names),
            lowering_input_output_aliases=(),
            sim_require_finite=True,
            sim_require_nnan=True,
            nc=nc,
        ))

    devices = jax.devices()[:NCORES]
    mesh = Mesh(np.asarray(devices), ("core",))
    in_specs = tuple(
        PartitionSpec("core") if nm in sharded_in else PartitionSpec()
        for nm in in_names) + (PartitionSpec("core"),) * len(out_names)
    sharded = jax.jit(
        shard_map(_body, mesh=mesh, in_specs=in_specs,
                  out_specs=(PartitionSpec("core"),) * len(out_names),
                  check_rep=False),
        donate_argnums=donate, keep_unused=True)

    def run(in_maps):
        args = []
        for nm in in_names:
            if nm in sharded_in:
                args.append(np.concatenate(
                    [np.asarray(in_maps[c][nm]) for c in range(NCORES)],
                    axis=0))
            else:
                args.append(np.asarray(in_maps[0][nm]))
        out_arrs = sharded(*args, *zero_outs)
        results = []
        for c in range(NCORES):
            results.append({
                nm: np.asarray(out_arrs[i]).reshape(
                    NCORES, *out_avals[i].shape)[c]
                for i, nm in enumerate(out_names)
            })
        return results

    return run


def kernel(**inputs):
    import sys
    if "/opt/trn_rl_repo" not in sys.path:
        sys.path.insert(0, "/opt/trn_rl_repo")
    from concourse.bass_utils import run_bass_kernel_spmd

    (xT, x_nat, Wf_dev, Wfsum, rwT, gb, sel, ident, eyerep, gb32,
     bet12) = _host_prep(inputs)

    if "nc" not in _CACHED:
        _CACHED["nc"] = _build_bass()
    nc = _CACHED["nc"]

    in_maps = []
    for c in range(NCORES):
        in_maps.append({
            "xT": xT[c],
            "xnat": x_nat[c],
            "wf": Wf_dev,
            "wfs": Wfsum,
            "rwT": rwT,
            "sel": sel,
            "ident": ident,
            "eyerep": eyerep,
            "gb32": gb32,
            "bet12": bet12,
        })
    if "runner" in _CACHED:
        results = _CACHED["runner"](in_maps)
    else:
        # first call goes through the standard entry point (compiles the
        # NEFF); subsequent calls reuse a cached jit executor
        res = run_bass_kernel_spmd(nc, in_maps,
                                   core_ids=list(range(NCORES)))
        results = res.results
        try:
            _CACHED["runner"] = _make_runner(nc)
        except Exception:
            pass
    full = np.empty((N, OC, T, V), np.float32)
    for c in range(NCORES):
        rc = results[c]
        for hp in range(H // 2):
            for tm in range(4):
                blk = rc[f"out_{hp}_{tm}"]        # [2, NS, 32, 32, V]
                for g in range(2):
                    full[c * NS:(c + 1) * NS,
                         32 * (2 * hp + g):32 * (2 * hp + g + 1),
                         32 * tm:32 * (tm + 1), :] = blk[g]
    return full


# revision 7
# speedup vs baseline: 1.1637x; 1.1637x over previous
"""Trainium2 Bass kernel for nn_BlockGC (gnn_message_passing) — v2.

Sharding: data-parallel over batch N=16 across 8 NeuronCores (2/core).
Exact BatchNorm batch stats via one tiny AllReduce.

Key layout/engine decisions (vs v1 baseline):
 - Host pre-packs x into the two SBUF layouts the kernel needs, so the
   device does 3 big contiguous DMAs (xT, x_nat, wf) instead of ~260
   small strided ones.
 - Main GEMM per (head h, sample n): PSUM[t, (w,o')] accumulated over 4
   contraction chunks of (c', v)=512; channels stay fused per head
   (Wf = wg*BnA fused on host).
 - BN stats on PE: Σval via ones-column matmuls over the bf16 value
   tiles; residual Σval via xsum; residual Σval² via Gram diagonal
   (G = RᵀR accumulated on PE, diag extracted with an identity mask).
 - Combine phase runs in a (tm, o')-partition layout produced by DVE
   32x32 block transposes, so BN coefficients are per-partition scalars
   and the whole affine+relu is 2 DVE passes + 1 Act pass per head.
 - Output DMA writes (ts, w)=3200B contiguous runs per partition.
"""

import numpy as np

N, C, T, V = 16, 128, 128, 25
K, H, OC = 3, 8, 256
EPS_BN = 1e-5
EPS_NORM = 1e-4
NCORES = 8
NS = N // NCORES          # samples per core
CH = C // H               # 16
OCH = OC // H             # 32
VP = 32                   # padded V
M_FREE = OCH * V          # 800 = (o', w) free block per head
NTOT = N * T * V          # batchnorm sample count per channel

_CACHED = {}


def _host_prep(inputs):
    import ml_dtypes
    bf16 = ml_dtypes.bfloat16

    x = np.asarray(inputs["x"], np.float32)
    hop = np.asarray(inputs["hop"])
    emb = np.asarray(inputs["emb_table"], np.float32)
    A = np.asarray(inputs["A"], np.float32)
    w_block = np.asarray(inputs["w_block"], np.float32)
    res_w = np.asarray(inputs["res_w"], np.float32)

    B = emb[:, :, hop]                                  # [K,H,V,V]

    def coln(w):
        return np.sqrt((w * w).sum(axis=-2, keepdims=True)) + EPS_NORM

    BnA = B / coln(B) + A / coln(A)                     # [K,H,V,V]

    wg = w_block.reshape(K, H, OCH, CH)                 # [K,H,o',c']
    # fused weight, free dim in (w, o') order to match fin layout
    Wf = np.einsum("khoc,khvw->hcvwo", wg, BnA)         # [H,CH,V,V,OCH]
    Wf_p = np.zeros((H, CH, VP, V, OCH), np.float32)
    Wf_p[:, :, :V] = Wf
    # contraction rows: c' = 4r + a ; partition p = 32a + v
    Wf_dev = Wf_p.reshape(H, 4, 4, VP, M_FREE)          # [H,r,a,v,(w o)]
    Wf_dev = np.ascontiguousarray(
        Wf_dev.reshape(H, 4, 128, M_FREE).astype(bf16))
    # w-summed weights for the Σval shortcut: Σ_w Wf[.., w, o]
    Wfsum = Wf_p.sum(axis=3)                             # [H,CH,VP,OCH]
    Wfsum = np.ascontiguousarray(
        Wfsum.reshape(H, 4, 128, OCH).astype(bf16))

    xb = x.astype(bf16)                                  # [N,C,T,V]
    xp = np.zeros((N, C, T, VP), bf16)
    xp[..., :V] = xb

    # x_nat: [core][c, n, t, v32] contiguous
    x_nat = np.ascontiguousarray(
        xp.reshape(NCORES, NS, C, T, VP).transpose(0, 2, 1, 3, 4))

    # xT: [core][p=(a,v32), h, r, n, t] contiguous
    #   c = 16h + 4r + a
    x6 = xp.reshape(NCORES, NS, H, 4, 4, T, VP)          # [co,n,h,r,a,t,v]
    xT = np.ascontiguousarray(x6.transpose(0, 4, 6, 2, 3, 1, 5)) \
        .reshape(NCORES, 128, H, 4, NS, T)               # [(a v),h,r,n,t]

    rwT = np.ascontiguousarray(res_w.T.astype(bf16))     # [C, OC]

    gb = np.ascontiguousarray(np.concatenate([
        np.asarray(inputs["bn_gamma"], np.float32),
        np.asarray(inputs["bn_beta"], np.float32),
        np.asarray(inputs["res_bn_gamma"], np.float32),
        np.asarray(inputs["res_bn_beta"], np.float32),
    ])[None, :])                                         # [1, 4*256]

    sel = np.zeros((32, 128), np.float32)                # selector for coef
    for p in range(128):
        sel[p % 32, p] = 1.0
    ident = np.ascontiguousarray(np.eye(128, dtype=np.float32))

    # one-hot column blocks: eyerep[:, 8h+j] = (j == h), for stat matmuls
    eyerep = np.zeros((128, H * H), bf16)
    for h in range(H):
        eyerep[:, H * h + h] = 1.0

    # per-partition-o' gamma / beta12 for the [32, *] coef math
    g1 = np.asarray(inputs["bn_gamma"], np.float32).reshape(H, OCH)
    g2 = np.asarray(inputs["res_bn_gamma"], np.float32).reshape(H, OCH)
    b1 = np.asarray(inputs["bn_beta"], np.float32).reshape(H, OCH)
    b2 = np.asarray(inputs["res_bn_beta"], np.float32).reshape(H, OCH)
    gb32 = np.ascontiguousarray(
        np.stack([g1, g2], axis=0).transpose(2, 0, 1))   # [32, 2, 8]
    bet12 = np.ascontiguousarray((b1 + b2).T)            # [32, 8]

    return (xT, x_nat, Wf_dev, Wfsum, rwT, gb, sel, ident, eyerep, gb32,
            bet12)


# ---------------------------------------------------------------------------
# Post-pass: this walrus build only accepts ONE sync wait / update command per
# instruction.  Split excess waits onto NOPs inserted before the instruction
# (same engine), excess updates onto NOPs after it.
# ---------------------------------------------------------------------------
def _split_excess_sync(nc, max_waits=1, max_updates=1):
    import bass_rust
    import concourse.mybir as mybir

    eng_map = None

    def make_nop(engine):
        nonlocal eng_map
        if eng_map is None:
            eng_map = {
                mybir.EngineType.SP: nc.sync,
                mybir.EngineType.DVE: nc.vector,
                mybir.EngineType.Activation: nc.scalar,
                mybir.EngineType.PE: nc.tensor,
                mybir.EngineType.Pool: nc.gpsimd,
            }
        bi = eng_map[engine].nop()
        inst = bi.ins
        f = nc.m.functions[0]
        for bb in f.blocks:
            names = [i.name for i in bb.instructions]
            if inst.name in names:
                lst = list(bb.instructions)
                lst.pop(names.index(inst.name))
                bb.instructions = lst
                break
        return inst

    f = nc.m.functions[0]
    for bb in f.blocks:
        insts = list(bb.instructions)
        out = []
        changed = False
        for inst in insts:
            si = inst.sync_info
            waits = list(si.on_wait) if si and si.on_wait else []
            ups = list(si.on_update) if si and si.on_update else []
            if len(waits) > max_waits:
                excess = waits[:-max_waits]
                keep = waits[-max_waits:]
                for i in range(0, len(excess), max_waits):
                    nop = make_nop(inst.engine)
                    nop.sync_info = bass_rust.SyncInfo(
                        on_wait=excess[i:i + max_waits], on_update=[])
                    out.append(nop)
                inst.sync_info = bass_rust.SyncInfo(on_wait=keep, on_update=ups)
                changed = True
            out.append(inst)
            if len(ups) > max_updates:
                keep_u = ups[:max_updates]
                excess_u = ups[max_updates:]
                si2 = inst.sync_info
                inst.sync_info = bass_rust.SyncInfo(
                    on_wait=list(si2.on_wait or []), on_update=keep_u)
                for i in range(0, len(excess_u), max_updates):
                    nop = make_nop(inst.engine)
                    nop.sync_info = bass_rust.SyncInfo(
                        on_wait=[], on_update=excess_u[i:i + max_updates])
                    out.append(nop)
                changed = True
        if changed:
            bb.instructions = out


def _build_bass():
    import concourse.bass as bass
    import concourse.mybir as mybir
    import concourse.tile as tile

    f32 = mybir.dt.float32
    bf16 = mybir.dt.bfloat16
    Alu = mybir.AluOpType
    Act = mybir.ActivationFunctionType

    nc = bass.Bass(num_devices=NCORES)

    xTp = nc.declare_dram_parameter("xT", [128, H, 4, NS, T], bf16,
                                    isOutput=False)
    xnp = nc.declare_dram_parameter("xnat", [128, NS, T, VP], bf16,
                                    isOutput=False)
    wfp = nc.declare_dram_parameter("wf", [H, 4, 128, M_FREE], bf16,
                                    isOutput=False)
    wfsp = nc.declare_dram_parameter("wfs", [H, 4, 128, OCH], bf16,
                                     isOutput=False)
    rwp = nc.declare_dram_parameter("rwT", [C, OC], bf16, isOutput=False)
    selp = nc.declare_dram_parameter("sel", [32, 128], f32, isOutput=False)
    idp = nc.declare_dram_parameter("ident", [128, 128], f32, isOutput=False)
    eyep = nc.declare_dram_parameter("eyerep", [128, H * H], bf16,
                                     isOutput=False)
    g32p = nc.declare_dram_parameter("gb32", [32, 2, H], f32, isOutput=False)
    b12p = nc.declare_dram_parameter("bet12", [32, H], f32, isOutput=False)
    # output split into one param per (head-pair, tm) so the store DMAs
    # don't WAW-chain on a single DRAM tensor; host reassembles.
    outs = [
        nc.declare_dram_parameter(f"out_{hp}_{tm}", [2, NS, 32, 32, V], bf16,
                                  isOutput=True)
        for hp in range(H // 2) for tm in range(4)
    ]

    cc_in = nc.dram_tensor("cc_in", [1, 4 * OC], f32)
    cc_out = nc.dram_tensor("cc_out", [1, 4 * OC], f32, addr_space="Shared")

    with tile.TileContext(nc) as tc:
        with (
            tc.tile_pool(name="vals", bufs=1) as p_vals,
            tc.tile_pool(name="small", bufs=1) as p_small,
        ):
            # ---- small constants ----
            rw_sb = p_small.tile([128, OC], bf16, tag="rw")
            nc.sync.dma_start(rw_sb[:], rwp[:])
            sel_sb = p_small.tile([32, 128], f32, tag="sel")
            nc.sync.dma_start(sel_sb[:], selp[:])
            id_sb = p_small.tile([128, 128], f32, tag="ident")
            nc.sync.dma_start(id_sb[:], idp[:])
            ones_b = p_small.tile([128, 1], bf16, tag="onesb")
            nc.vector.memset(ones_b[:], 1.0)
            ones_f = p_small.tile([128, 1], f32, tag="onesf")
            nc.vector.memset(ones_f[:], 1.0)
            eye_sb = p_small.tile([128, H * H], bf16, tag="eyerep")
            nc.sync.dma_start(eye_sb[:], eyep[:])
            g32_sb = p_small.tile([32, 2, H], f32, tag="gb32")
            nc.sync.dma_start(g32_sb[:], g32p[:])
            b12_sb = p_small.tile([32, H], f32, tag="bet12")
            nc.sync.dma_start(b12_sb[:], b12p[:])
            eps_ap = p_small.tile([32, 1], f32, tag="eps")
            nc.vector.memset(eps_ap[:], EPS_BN)

            # ---- value tensors (outlive GEMM-phase pools) ----
            # fin: [p=t(tm,ts), n, w, h, o']
            fin = p_vals.tile([128, NS, V, H, OCH], bf16, tag="fin")
            # res_fin: [p=t, n, w, (h o')=oc]
            res_fin = p_vals.tile([128, NS, V, OC], bf16, tag="resfin")
            # val stat rows [mval 256 | rval 256] and residual sq row
            valrow = p_vals.tile([1, 512], f32, tag="valrow")
            resrow = p_vals.tile([1, 256], f32, tag="resrow")

            with (
                tc.tile_pool(name="xbig", bufs=1) as p_x,
                tc.tile_pool(name="scr", bufs=2) as p_scr,
            ):
                xT_sb = p_x.tile([128, H, 4, NS, T], bf16, tag="xT")
                nc.sync.dma_start(xT_sb[:], xTp[:])
                wf_sb = p_x.tile([128, H, 4, M_FREE], bf16, tag="wf")
                for h in range(H):
                    nc.sync.dma_start(wf_sb[:, h],
                                      wfp[h].rearrange("r p m -> p r m"))
                x_nat = p_x.tile([128, NS, T, VP], bf16, tag="xnat")
                nc.sync.dma_start(x_nat[:], xnp[:])
                wfs_sb = p_x.tile([128, H, 4, OCH], bf16, tag="wfs")
                nc.sync.dma_start(wfs_sb[:],
                                  wfsp[:].rearrange("h r p m -> p h r m"))

                # Σ_{n,t} x in both layouts (for the Σval shortcuts)
                xsm = p_x.tile([128, H, 4], f32, tag="xsm")
                nc.vector.reduce_sum(
                    xsm[:], xT_sb[:].rearrange("p h r n t -> p h r (n t)"),
                    axis=mybir.AxisListType.X)
                xsm_b = p_x.tile([128, H, 4], bf16, tag="xsmb")
                nc.vector.tensor_copy(xsm_b[:], xsm[:])
                xsn = p_x.tile([128, 1], f32, tag="xsn")
                nc.vector.reduce_sum(
                    xsn[:], x_nat[:].rearrange("p n t v -> p (n t v)"),
                    axis=mybir.AxisListType.X)
                xsn_b = p_x.tile([128, 1], bf16, tag="xsnb")
                nc.vector.tensor_copy(xsn_b[:], xsn[:])

                # ---------------- main fused GEMMs + stats ----------------
                with (
                    tc.tile_pool(name="pm", bufs=2, space="PSUM") as p_pm,
                    tc.tile_pool(name="pst", bufs=1, space="PSUM") as p_pst,
                ):
                    # sq stat rows accumulate across ALL heads: row h of the
                    # [8, 800] psum receives head h's Σ_t via a one-hot
                    # column stationary (eyerep[:, 8h:8h+8]).
                    ps_sq = p_pst.tile([8, M_FREE], f32, tag="pssq")
                    for h in range(H):
                        for n in range(NS):
                            pm = p_pm.tile([128, M_FREE], f32, tag="pmain")
                            for r in range(4):
                                st, sp = (r == 0), (r == 3)
                                nc.tensor.matmul(pm[:, 0:512],
                                                 xT_sb[:, h, r, n, :],
                                                 wf_sb[:, h, r, 0:512],
                                                 start=st, stop=sp)
                                nc.tensor.matmul(pm[:, 512:M_FREE],
                                                 xT_sb[:, h, r, n, :],
                                                 wf_sb[:, h, r, 512:M_FREE],
                                                 start=st, stop=sp)
                            # evict values (bf16), fin free = (w, o')
                            fv = fin[:, n, :, h, :]
                            nc.scalar.activation(
                                fv, pm[:, 0:M_FREE].rearrange(
                                    "p (w o) -> p w o", w=V, o=OCH),
                                Act.Copy)
                            # squares (bf16, 2x DVE)
                            scr = p_scr.tile([128, V, OCH], bf16, tag="sq")
                            nc.vector.tensor_mul(scr[:], fv, fv)
                            # sq stat matmuls: Σ_t into row h of ps_sq
                            st = (h == 0 and n == 0)
                            sp = (h == H - 1 and n == NS - 1)
                            eh = eye_sb[:, H * h:H * (h + 1)]
                            nc.tensor.matmul(ps_sq[:, 0:512], eh,
                                             scr[:, 0:16, :],
                                             start=st, stop=sp)
                            nc.tensor.matmul(ps_sq[:, 512:M_FREE], eh,
                                             scr[:, 16:V, :],
                                             start=st, stop=sp)

                    # Σval via the w-summed weights: val[h,o'] =
                    #   Σ_r xsumᵀ(h,r) · Wfsum(h,r)
                    ps_mv = p_pst.tile([1, OC], f32, tag="psmv")
                    for h in range(H):
                        for r in range(4):
                            nc.tensor.matmul(ps_mv[:, 32 * h:32 * (h + 1)],
                                             xsm_b[:, h, r:r + 1],
                                             wfs_sb[:, h, r, :],
                                             start=(r == 0), stop=(r == 3))
                    nc.scalar.activation(valrow[:, 0:OC], ps_mv[:], Act.Copy)

                    # evict sq rows, reduce over w, ship to the AR input
                    mrow = p_scr.tile([8, M_FREE], f32, tag="mrow")
                    nc.scalar.activation(mrow[:], ps_sq[:], Act.Copy)
                    mred = p_scr.tile([8, OCH], f32, tag="mred")
                    nc.vector.reduce_sum(
                        mred[:],
                        mrow[:].rearrange("p (w o) -> p o w", w=V, o=OCH),
                        axis=mybir.AxisListType.X)
                    nc.sync.dma_start(cc_in[:, 512:768], mred[:])

                # -------- residual GEMMs + stats (interleaved) --------
                # Per (n, v): GEMM -> evict (Pool) -> Σval ones-matmul and
                # Gram accumulation ride in the PE stream right behind.
                with (
                    tc.tile_pool(name="pr", bufs=4, space="PSUM") as p_pr,
                    tc.tile_pool(name="prs", bufs=1, space="PSUM") as p_prs,
                ):
                    ps_rv = p_prs.tile([1, OC], f32, tag="psrv")
                    gps0 = p_prs.tile([128, 128], f32, tag="gram0")
                    gps1 = p_prs.tile([128, 128], f32, tag="gram1")
                    # Σ res values = xsum_natᵀ · rw
                    nc.tensor.matmul(ps_rv[:], xsn_b[:], rw_sb[:],
                                     start=True, stop=True)
                    nc.scalar.activation(valrow[:, OC:2 * OC], ps_rv[:],
                                         Act.Copy)
                    for n in range(NS):
                        for v in range(V):
                            pr = p_pr.tile([128, OC], f32, tag="pres")
                            nc.tensor.matmul(pr[:], x_nat[:, n, :, v],
                                             rw_sb[:], start=True, stop=True)
                            # GPSIMD can't read PSUM on HW; split evictions
                            # between Act and DVE
                            if v % 2 == 0:
                                nc.scalar.activation(res_fin[:, n, v, :],
                                                     pr[:], Act.Copy)
                            else:
                                nc.vector.tensor_copy(res_fin[:, n, v, :],
                                                      pr[:])
                            st = (n == 0 and v == 0)
                            sp = (n == NS - 1 and v == V - 1)
                            for c, gps in ((0, gps0), (1, gps1)):
                                sl = res_fin[:, n, v, 128 * c:128 * (c + 1)]
                                nc.tensor.matmul(gps[:], sl, sl,
                                                 start=st, stop=sp)
                    # Σ res² = diag(Gram)
                    for c, gps in ((0, gps0), (1, gps1)):
                        dsb = p_scr.tile([128, 128], f32, tag="diag")
                        nc.vector.tensor_mul(dsb[:], gps[:], id_sb[:])
                        ps_d = p_prs.tile([1, 128], f32, tag="psd")
                        nc.tensor.matmul(ps_d[:], ones_f[:], dsb[:],
                                         start=True, stop=True)
                        nc.scalar.activation(
                            resrow[:, 128 * c:128 * (c + 1)],
                            ps_d[:], Act.Copy)

            # ---------------- AllReduce ----------
            # AR payload: [mval 256 | rval 256 | msq 256 | rsq 256]
            # (mred -> [512:768] shipped above; spread DMAs across engines)
            nc.scalar.dma_start(cc_in[:, 0:512], valrow[:])
            nc.gpsimd.dma_start(cc_in[:, 768:1024], resrow[:])
            nc.gpsimd.collective_compute(
                "AllReduce", Alu.add,
                replica_groups=[list(range(NCORES))],
                ins=[cc_in[:]], outs=[cc_out[:]])
            # load reduced stats partition-spread: statg32[o', kind, h]
            statg = p_small.tile([32, 4, H], f32, tag="statg")
            nc.sync.dma_start(
                statg[:],
                cc_out[:].rearrange("one (k h o) -> (one o) k h",
                                    k=4, h=H, o=OCH))

            # ---------------- transposed (channel-major) values ---------
            with (
                tc.tile_pool(name="cm", bufs=1) as p_cm,
                tc.tile_pool(name="fo", bufs=2) as p_fo,
                tc.tile_pool(name="pc", bufs=1, space="PSUM") as p_pc,
            ):
                # main_cm/res_cm: [p=(tm,o'), n, w, h, ts]
                main_cm = p_cm.tile([128, NS, V, H, 32], bf16, tag="mcm")
                res_cm = p_cm.tile([128, NS, V, H, 32], bf16, tag="rcm")
                for h in range(H):
                    nc.vector.transpose(main_cm[:, :, :, h, :],
                                        fin[:, :, :, h, :])
                    nc.vector.transpose(
                        res_cm[:, :, :, h, :],
                        res_fin[:, :, :, :].rearrange(
                            "p n w (h o) -> p n w h o", h=H, o=OCH)[:, :, :, h, :])

                # ---------------- coefficients ----------------
                # statg [32, kind, h]: kinds (mval, rval, msq, rsq).
                # Everything on 32 partitions (one per o').
                coef32 = p_small.tile([32, 3, H], f32, tag="coef32")
                AB_v = coef32[:, 0:2, :]
                E_v = coef32[:, 2, :]
                mu = p_small.tile([32, 2, H], f32, tag="cmu")
                mu2 = p_small.tile([32, 2, H], f32, tag="cmu2")
                inv = 1.0 / float(NTOT)

                nc.vector.tensor_scalar_mul(mu[:], statg[:, 0:2, :], inv)
                nc.vector.tensor_mul(mu2[:], mu[:], mu[:])
                # var = sq/N - mu^2
                nc.vector.scalar_tensor_tensor(
                    AB_v, statg[:, 2:4, :], inv, mu2[:],
                    Alu.mult, Alu.subtract)
                # sd = sqrt(var + eps); A,B = gamma / sd
                nc.scalar.activation(AB_v, AB_v, Act.Sqrt, bias=eps_ap[:])
                nc.vector.reciprocal(AB_v, AB_v)
                nc.vector.tensor_mul(AB_v, AB_v, g32_sb[:])
                # E = (b1+b2) - A*mu_m - B*mu_r
                nc.vector.tensor_mul(mu2[:], AB_v, mu[:])
                nc.vector.tensor_sub(E_v, b12_sb[:], mu2[:, 0, :])
                nc.vector.tensor_sub(E_v, E_v, mu2[:, 1, :])

                # broadcast per-o' coef rows to all 128 partitions:
                # cb[p, (c,h)] = coef32[p % 32, c, h]
                cb_ps = p_pc.tile([128, 3 * H], f32, tag="cbps")
                nc.tensor.matmul(cb_ps[:], sel_sb[:],
                                 coef32[:].rearrange("o c h -> o (c h)"),
                                 start=True, stop=True)
                coef = p_small.tile([128, 3 * H], f32, tag="coef")
                nc.vector.tensor_copy(coef[:], cb_ps[:])

                # ---------------- combine + relu + out DMA ----------------
                # head-pair pipeline; each (head-pair, tm) block ships to
                # its own DRAM param with (ts,w)=3200B contiguous runs.
                # Engine assignment keeps SP/Pool even and gives Act the
                # tail DMAs (after its relus are done).
                dma_q = [nc.sync, nc.gpsimd, nc.sync, nc.gpsimd,
                         nc.sync, nc.gpsimd, nc.sync, nc.gpsimd,
                         nc.sync, nc.gpsimd, nc.scalar, nc.scalar,
                         nc.sync, nc.gpsimd, nc.scalar, nc.scalar]
                for h in range(H):
                    A_h = coef[:, h:h + 1]
                    B_h = coef[:, H + h:H + h + 1]
                    E_h = coef[:, 2 * H + h:2 * H + h + 1]
                    t1 = p_fo.tile([128, NS, V, 32], bf16, tag="t1")
                    nc.vector.tensor_scalar(t1[:], res_cm[:, :, :, h, :],
                                            B_h, E_h, Alu.mult, Alu.add)
                    t2b = p_fo.tile([128, NS, V, 32], bf16, tag="t2")
                    nc.vector.scalar_tensor_tensor(
                        t2b[:], main_cm[:, :, :, h, :], A_h, t1[:],
                        Alu.mult, Alu.add)
                    if h % 2 == 0:
                        fo2 = p_fo.tile([128, 2, NS, 32, V], bf16, tag="fo")
                    nc.scalar.activation(
                        fo2[:, h % 2].rearrange("p n ts w -> p n w ts"),
                        t2b[:], Act.Relu)
                    if h % 2 == 1:
                        hp = h // 2
                        for tm in range(4):
                            dma_q[4 * hp + tm].dma_start(
                                outs[4 * hp + tm][:].rearrange(
                                    "g n o ts w -> o (g n) (ts w)"),
                                fo2[32 * tm:32 * (tm + 1)].rearrange(
                                    "p g n ts w -> p (g n) (ts w)"))

    _split_excess_sync(nc)
    return nc


def _make_runner(nc):
    """Build a cached PJRT executor (same lowering path run_bass_kernel_spmd
    uses under axon, but the jit closure is built once so warm calls skip
    re-trace/re-lower)."""
    import jax
    import jax.numpy as jnp
    from jax.sharding import Mesh, PartitionSpec
    from jax.experimental.shard_map import shard_map
    from concourse import bass2jax
    from concourse import mybir

    import jax.numpy as jnp

    bass2jax.install_neuronx_cc_hook()
    partition_name = (nc.partition_id_tensor.name
                      if nc.partition_id_tensor else None)
    # per-core (sharded) vs replicated inputs
    sharded_in = {"xT", "xnat"}
    in_names, out_names, out_avals, zero_outs = [], [], [], []
    for alloc in nc.m.functions[0].allocations:
        if not isinstance(alloc, mybir.MemoryLocationSet):
            continue
        name = alloc.memorylocations[0].name
        if alloc.kind == "ExternalInput":
            if name != partition_name:
                in_names.append(name)
        elif alloc.kind == "ExternalOutput":
            shape = tuple(alloc.tensor_shape)
            dtype = mybir.dt.np(alloc.dtype)
            out_names.append(name)
            out_avals.append(jax.core.ShapedArray(shape, dtype))
            zero_outs.append(np.zeros((NCORES * shape[0], *shape[1:]), dtype))
    n_params = len(in_names)
    all_names = list(in_names) + list(out_names)
    if partition_name is not None:
        all_names.append(partition_name)
    donate = tuple(range(n_params, n_params + len(out_names)))

    def _body(*args):
        operands = list(args)
        if partition_name is not None:
            operands.append(bass2jax.partition_id_tensor())
        return tuple(bass2jax._bass_exec_p.bind(
            *operands,
            out_avals=tuple(out_avals),
            in_names=tuple(all_names),
            out_names=tuple(out_names),
            lowering_input_output_aliases=(),
            sim_require_finite=True,
            sim_require_nnan=True,
            nc=nc,
        ))

    devices = jax.devices()[:NCORES]
    mesh = Mesh(np.asarray(devices), ("core",))
    in_specs = tuple(
        PartitionSpec("core") if nm in sharded_in else PartitionSpec()
        for nm in in_names) + (PartitionSpec("core"),) * len(out_names)
    sharded = jax.jit(
        shard_map(_body, mesh=mesh, in_specs=in_specs,
                  out_specs=(PartitionSpec("core"),) * len(out_names),
                  check_rep=False),
        donate_argnums=donate, keep_unused=True)

    def run(in_maps):
        args = []
        for nm in in_names:
            if nm in sharded_in:
                args.append(np.concatenate(
                    [np.asarray(in_maps[c][nm]) for c in range(NCORES)],
                    axis=0))
            else:
                args.append(np.asarray(in_maps[0][nm]))
        out_arrs = sharded(*args, *zero_outs)
        results = []
        for c in range(NCORES):
            results.append({
                nm: np.asarray(out_arrs[i]).reshape(
                    NCORES, *out_avals[i].shape)[c]
                for i, nm in enumerate(out_names)
            })
        return results

    return run


def kernel(**inputs):
    import sys
    if "/opt/trn_rl_repo" not in sys.path:
        sys.path.insert(0, "/opt/trn_rl_repo")
    from concourse.bass_utils import run_bass_kernel_spmd

    (xT, x_nat, Wf_dev, Wfsum, rwT, gb, sel, ident, eyerep, gb32,
     bet12) = _host_prep(inputs)

    if "nc" not in _CACHED:
        _CACHED["nc"] = _build_bass()
    nc = _CACHED["nc"]

    in_maps = []
    for c in range(NCORES):
        in_maps.append({
            "xT": xT[c],
            "xnat": x_nat[c],
            "wf": Wf_dev,
            "wfs": Wfsum,
            "rwT": rwT,
            "sel": sel,
            "ident": ident,
            "eyerep": eyerep,
            "gb32": gb32,
            "bet12": bet12,
        })
    if "runner" in _CACHED:
        results = _CACHED["runner"](in_maps)
    else:
        # first call goes through the standard entry point (compiles the
        # NEFF); subsequent calls reuse a cached jit executor
        res = run_bass_kernel_spmd(nc, in_maps,
                                   core_ids=list(range(NCORES)))
        results = res.results
        try:
            _CACHED["runner"] = _make_runner(nc)
        except Exception:
            pass
    full = np.empty((N, OC, T, V), np.float32)
    for c in range(NCORES):
        rc = results[c]
        for hp in range(H // 2):
            for tm in range(4):
                blk = rc[f"out_{hp}_{tm}"]        # [2, NS, 32, 32, V] bf16
                for g in range(2):
                    full[c * NS:(c + 1) * NS,
                         32 * (2 * hp + g):32 * (2 * hp + g + 1),
                         32 * tm:32 * (tm + 1), :] = \
                        blk[g].astype(np.float32)
    return full


# revision 8
# speedup vs baseline: 1.2206x; 1.0489x over previous
"""Trainium2 Bass kernel for nn_BlockGC (gnn_message_passing) — v2.

Sharding: data-parallel over batch N=16 across 8 NeuronCores (2/core).
Exact BatchNorm batch stats via one tiny AllReduce.

Key layout/engine decisions (vs v1 baseline, 341.6us -> 126.1us CoreSim):
 - Host pre-packs x into the two SBUF layouts the kernel needs, so the
   device does 3 big contiguous loads (xT, x_nat, wf) instead of ~260
   small strided DMAs (each DMA costs ~0.6us HWDGE + dispatch).
 - Main GEMM per (head h, sample n): PSUM[t, (w,o')] accumulated over 4
   contraction chunks of (c', v)=512; graph conv + grouped conv +
   K-subset sum stay fused per head (Wf = wg*BnA fused on host).
 - BN stats mostly on PE, off the DVE/Act critical path:
     Σval(main) = xsumᵀ·Wfsum (host pre-sums Wf over w),
     Σval(res)  = xsum_natᵀ·rw,
     Σval²(main) via one-hot-column matmuls accumulating all 8 heads
     into one [8, 800] PSUM tile, Σval²(res) via the Gram diagonal
     (G = RᵀR on PE, diag extracted with an identity mask).
 - Coefficient math runs [32, ·]-partition-spread directly from the AR
   result; a selector matmul broadcasts per-o' rows to 128 partitions.
 - Combine phase runs in a (tm, o')-partition layout produced by DVE
   32x32 block transposes (hidden under the AllReduce), so BN coeffs
   are per-partition scalars: 2 fused DVE passes (bf16 2x) + 1 Act
   relu pass per head.
 - Output ships bf16 (host upcasts) as 16 unchained DMAs — one DRAM
   param per (head-pair, tm) — with (ts,w) contiguous runs, spread
   over the SP/Pool/Act queues and pipelined with the combine.
"""

import numpy as np

N, C, T, V = 16, 128, 128, 25
K, H, OC = 3, 8, 256
EPS_BN = 1e-5
EPS_NORM = 1e-4
NCORES = 8
NS = N // NCORES          # samples per core
CH = C // H               # 16
OCH = OC // H             # 32
VP = 32                   # padded V
M_FREE = OCH * V          # 800 = (o', w) free block per head
NTOT = N * T * V          # batchnorm sample count per channel

_CACHED = {}


def _host_prep(inputs):
    import ml_dtypes
    bf16 = ml_dtypes.bfloat16

    x = np.asarray(inputs["x"], np.float32)
    hop = np.asarray(inputs["hop"])
    emb = np.asarray(inputs["emb_table"], np.float32)
    A = np.asarray(inputs["A"], np.float32)
    w_block = np.asarray(inputs["w_block"], np.float32)
    res_w = np.asarray(inputs["res_w"], np.float32)

    B = emb[:, :, hop]                                  # [K,H,V,V]

    def coln(w):
        return np.sqrt((w * w).sum(axis=-2, keepdims=True)) + EPS_NORM

    BnA = B / coln(B) + A / coln(A)                     # [K,H,V,V]

    wg = w_block.reshape(K, H, OCH, CH)                 # [K,H,o',c']
    # fused weight, free dim in (w, o') order to match fin layout
    Wf = np.einsum("khoc,khvw->hcvwo", wg, BnA)         # [H,CH,V,V,OCH]
    Wf_p = np.zeros((H, CH, VP, V, OCH), np.float32)
    Wf_p[:, :, :V] = Wf
    # contraction rows: c' = 4r + a ; partition p = 32a + v
    Wf_dev = Wf_p.reshape(H, 4, 4, VP, M_FREE)          # [H,r,a,v,(w o)]
    Wf_dev = np.ascontiguousarray(
        Wf_dev.reshape(H, 4, 128, M_FREE).astype(bf16))
    # w-summed weights for the Σval shortcut: Σ_w Wf[.., w, o]
    Wfsum = Wf_p.sum(axis=3)                             # [H,CH,VP,OCH]
    Wfsum = np.ascontiguousarray(
        Wfsum.reshape(H, 4, 128, OCH).astype(bf16))

    xb = x.astype(bf16)                                  # [N,C,T,V]
    xp = np.zeros((N, C, T, VP), bf16)
    xp[..., :V] = xb

    # x_nat: [core][c, n, t, v32] contiguous
    x_nat = np.ascontiguousarray(
        xp.reshape(NCORES, NS, C, T, VP).transpose(0, 2, 1, 3, 4))

    # xT: [core][p=(a,v32), h, r, n, t] contiguous
    #   c = 16h + 4r + a
    x6 = xp.reshape(NCORES, NS, H, 4, 4, T, VP)          # [co,n,h,r,a,t,v]
    xT = np.ascontiguousarray(x6.transpose(0, 4, 6, 2, 3, 1, 5)) \
        .reshape(NCORES, 128, H, 4, NS, T)               # [(a v),h,r,n,t]

    rwT = np.ascontiguousarray(res_w.T.astype(bf16))     # [C, OC]

    gb = np.ascontiguousarray(np.concatenate([
        np.asarray(inputs["bn_gamma"], np.float32),
        np.asarray(inputs["bn_beta"], np.float32),
        np.asarray(inputs["res_bn_gamma"], np.float32),
        np.asarray(inputs["res_bn_beta"], np.float32),
    ])[None, :])                                         # [1, 4*256]

    sel = np.zeros((32, 128), np.float32)                # selector for coef
    for p in range(128):
        sel[p % 32, p] = 1.0
    ident = np.ascontiguousarray(np.eye(128, dtype=np.float32))

    # one-hot column blocks: eyerep[:, 8h+j] = (j == h), for stat matmuls
    eyerep = np.zeros((128, H * H), bf16)
    for h in range(H):
        eyerep[:, H * h + h] = 1.0

    # per-partition-o' gamma / beta12 for the [32, *] coef math
    g1 = np.asarray(inputs["bn_gamma"], np.float32).reshape(H, OCH)
    g2 = np.asarray(inputs["res_bn_gamma"], np.float32).reshape(H, OCH)
    b1 = np.asarray(inputs["bn_beta"], np.float32).reshape(H, OCH)
    b2 = np.asarray(inputs["res_bn_beta"], np.float32).reshape(H, OCH)
    gb32 = np.ascontiguousarray(
        np.stack([g1, g2], axis=0).transpose(2, 0, 1))   # [32, 2, 8]
    bet12 = np.ascontiguousarray((b1 + b2).T)            # [32, 8]

    return (xT, x_nat, Wf_dev, Wfsum, rwT, gb, sel, ident, eyerep, gb32,
            bet12)


# ---------------------------------------------------------------------------
# Post-pass: this walrus build only accepts ONE sync wait / update command per
# instruction.  Split excess waits onto NOPs inserted before the instruction
# (same engine), excess updates onto NOPs after it.
# ---------------------------------------------------------------------------
def _split_excess_sync(nc, max_waits=1, max_updates=1):
    import bass_rust
    import concourse.mybir as mybir

    eng_map = None

    def make_nop(engine):
        nonlocal eng_map
        if eng_map is None:
            eng_map = {
                mybir.EngineType.SP: nc.sync,
                mybir.EngineType.DVE: nc.vector,
                mybir.EngineType.Activation: nc.scalar,
                mybir.EngineType.PE: nc.tensor,
                mybir.EngineType.Pool: nc.gpsimd,
            }
        bi = eng_map[engine].nop()
        inst = bi.ins
        f = nc.m.functions[0]
        for bb in f.blocks:
            names = [i.name for i in bb.instructions]
            if inst.name in names:
                lst = list(bb.instructions)
                lst.pop(names.index(inst.name))
                bb.instructions = lst
                break
        return inst

    f = nc.m.functions[0]
    for bb in f.blocks:
        insts = list(bb.instructions)
        out = []
        changed = False
        for inst in insts:
            si = inst.sync_info
            waits = list(si.on_wait) if si and si.on_wait else []
            ups = list(si.on_update) if si and si.on_update else []
            if len(waits) > max_waits:
                excess = waits[:-max_waits]
                keep = waits[-max_waits:]
                for i in range(0, len(excess), max_waits):
                    nop = make_nop(inst.engine)
                    nop.sync_info = bass_rust.SyncInfo(
                        on_wait=excess[i:i + max_waits], on_update=[])
                    out.append(nop)
                inst.sync_info = bass_rust.SyncInfo(on_wait=keep, on_update=ups)
                changed = True
            out.append(inst)
            if len(ups) > max_updates:
                keep_u = ups[:max_updates]
                excess_u = ups[max_updates:]
                si2 = inst.sync_info
                inst.sync_info = bass_rust.SyncInfo(
                    on_wait=list(si2.on_wait or []), on_update=keep_u)
                for i in range(0, len(excess_u), max_updates):
                    nop = make_nop(inst.engine)
                    nop.sync_info = bass_rust.SyncInfo(
                        on_wait=[], on_update=excess_u[i:i + max_updates])
                    out.append(nop)
                changed = True
        if changed:
            bb.instructions = out


def _build_bass():
    import concourse.bass as bass
    import concourse.mybir as mybir
    import concourse.tile as tile

    f32 = mybir.dt.float32
    bf16 = mybir.dt.bfloat16
    Alu = mybir.AluOpType
    Act = mybir.ActivationFunctionType

    nc = bass.Bass(num_devices=NCORES)

    xTp = nc.declare_dram_parameter("xT", [128, H, 4, NS, T], bf16,
                                    isOutput=False)
    xnp = nc.declare_dram_parameter("xnat", [128, NS, T, VP], bf16,
                                    isOutput=False)
    wfp = nc.declare_dram_parameter("wf", [H, 4, 128, M_FREE], bf16,
                                    isOutput=False)
    wfsp = nc.declare_dram_parameter("wfs", [H, 4, 128, OCH], bf16,
                                     isOutput=False)
    rwp = nc.declare_dram_parameter("rwT", [C, OC], bf16, isOutput=False)
    selp = nc.declare_dram_parameter("sel", [32, 128], f32, isOutput=False)
    idp = nc.declare_dram_parameter("ident", [128, 128], f32, isOutput=False)
    eyep = nc.declare_dram_parameter("eyerep", [128, H * H], bf16,
                                     isOutput=False)
    g32p = nc.declare_dram_parameter("gb32", [32, 2, H], f32, isOutput=False)
    b12p = nc.declare_dram_parameter("bet12", [32, H], f32, isOutput=False)
    # output split into one param per (head-pair, tm) so the store DMAs
    # don't WAW-chain on a single DRAM tensor; host reassembles.
    outs = [
        nc.declare_dram_parameter(f"out_{hp}_{tm}", [2, NS, 32, 32, V], bf16,
                                  isOutput=True)
        for hp in range(H // 2) for tm in range(4)
    ]

    cc_in = nc.dram_tensor("cc_in", [1, 4 * OC], f32)
    cc_out = nc.dram_tensor("cc_out", [1, 4 * OC], f32, addr_space="Shared")

    with tile.TileContext(nc) as tc:
        with (
            tc.tile_pool(name="vals", bufs=1) as p_vals,
            tc.tile_pool(name="small", bufs=1) as p_small,
        ):
            # ---- small constants ----
            rw_sb = p_small.tile([128, OC], bf16, tag="rw")
            nc.sync.dma_start(rw_sb[:], rwp[:])
            sel_sb = p_small.tile([32, 128], f32, tag="sel")
            nc.sync.dma_start(sel_sb[:], selp[:])
            id_sb = p_small.tile([128, 128], f32, tag="ident")
            nc.sync.dma_start(id_sb[:], idp[:])
            ones_b = p_small.tile([128, 1], bf16, tag="onesb")
            nc.vector.memset(ones_b[:], 1.0)
            ones_f = p_small.tile([128, 1], f32, tag="onesf")
            nc.vector.memset(ones_f[:], 1.0)
            eye_sb = p_small.tile([128, H * H], bf16, tag="eyerep")
            nc.sync.dma_start(eye_sb[:], eyep[:])
            g32_sb = p_small.tile([32, 2, H], f32, tag="gb32")
            nc.sync.dma_start(g32_sb[:], g32p[:])
            b12_sb = p_small.tile([32, H], f32, tag="bet12")
            nc.sync.dma_start(b12_sb[:], b12p[:])
            eps_ap = p_small.tile([32, 1], f32, tag="eps")
            nc.vector.memset(eps_ap[:], EPS_BN)

            # ---- value tensors (outlive GEMM-phase pools) ----
            # fin: [p=t(tm,ts), n, w, h, o']
            fin = p_vals.tile([128, NS, V, H, OCH], bf16, tag="fin")
            # res_fin: [p=t, n, w, (h o')=oc]
            res_fin = p_vals.tile([128, NS, V, OC], bf16, tag="resfin")
            # val stat rows [mval 256 | rval 256] and residual sq row
            valrow = p_vals.tile([1, 512], f32, tag="valrow")
            resrow = p_vals.tile([1, 256], f32, tag="resrow")

            with (
                tc.tile_pool(name="xbig", bufs=1) as p_x,
                tc.tile_pool(name="scr", bufs=2) as p_scr,
            ):
                xT_sb = p_x.tile([128, H, 4, NS, T], bf16, tag="xT")
                nc.sync.dma_start(xT_sb[:], xTp[:])
                wf_sb = p_x.tile([128, H, 4, M_FREE], bf16, tag="wf")
                for h in range(H):
                    nc.sync.dma_start(wf_sb[:, h],
                                      wfp[h].rearrange("r p m -> p r m"))
                x_nat = p_x.tile([128, NS, T, VP], bf16, tag="xnat")
                nc.sync.dma_start(x_nat[:], xnp[:])
                wfs_sb = p_x.tile([128, H, 4, OCH], bf16, tag="wfs")
                nc.sync.dma_start(wfs_sb[:],
                                  wfsp[:].rearrange("h r p m -> p h r m"))

                # Σ_{n,t} x in both layouts (for the Σval shortcuts)
                xsm = p_x.tile([128, H, 4], f32, tag="xsm")
                nc.vector.reduce_sum(
                    xsm[:], xT_sb[:].rearrange("p h r n t -> p h r (n t)"),
                    axis=mybir.AxisListType.X)
                xsm_b = p_x.tile([128, H, 4], bf16, tag="xsmb")
                nc.vector.tensor_copy(xsm_b[:], xsm[:])
                xsn = p_x.tile([128, 1], f32, tag="xsn")
                nc.vector.reduce_sum(
                    xsn[:], x_nat[:].rearrange("p n t v -> p (n t v)"),
                    axis=mybir.AxisListType.X)
                xsn_b = p_x.tile([128, 1], bf16, tag="xsnb")
                nc.vector.tensor_copy(xsn_b[:], xsn[:])

                # ---------------- main fused GEMMs + stats ----------------
                with (
                    tc.tile_pool(name="pm", bufs=2, space="PSUM") as p_pm,
                    tc.tile_pool(name="pst", bufs=1, space="PSUM") as p_pst,
                ):
                    # sq stat rows accumulate across ALL heads: row h of the
                    # [8, 800] psum receives head h's Σ_t via a one-hot
                    # column stationary (eyerep[:, 8h:8h+8]).
                    ps_sq = p_pst.tile([8, M_FREE], f32, tag="pssq")
                    for h in range(H):
                        for n in range(NS):
                            pm = p_pm.tile([128, M_FREE], f32, tag="pmain")
                            for r in range(4):
                                st, sp = (r == 0), (r == 3)
                                nc.tensor.matmul(pm[:, 0:512],
                                                 xT_sb[:, h, r, n, :],
                                                 wf_sb[:, h, r, 0:512],
                                                 start=st, stop=sp)
                                nc.tensor.matmul(pm[:, 512:M_FREE],
                                                 xT_sb[:, h, r, n, :],
                                                 wf_sb[:, h, r, 512:M_FREE],
                                                 start=st, stop=sp)
                            # evict values (bf16), fin free = (w, o')
                            fv = fin[:, n, :, h, :]
                            nc.scalar.activation(
                                fv, pm[:, 0:M_FREE].rearrange(
                                    "p (w o) -> p w o", w=V, o=OCH),
                                Act.Copy)
                            # squares (bf16, 2x DVE)
                            scr = p_scr.tile([128, V, OCH], bf16, tag="sq")
                            nc.vector.tensor_mul(scr[:], fv, fv)
                            # sq stat matmuls: Σ_t into row h of ps_sq
                            st = (h == 0 and n == 0)
                            sp = (h == H - 1 and n == NS - 1)
                            eh = eye_sb[:, H * h:H * (h + 1)]
                            nc.tensor.matmul(ps_sq[:, 0:512], eh,
                                             scr[:, 0:16, :],
                                             start=st, stop=sp)
                            nc.tensor.matmul(ps_sq[:, 512:M_FREE], eh,
                                             scr[:, 16:V, :],
                                             start=st, stop=sp)

                    # Σval via the w-summed weights: val[h,o'] =
                    #   Σ_r xsumᵀ(h,r) · Wfsum(h,r)
                    ps_mv = p_pst.tile([1, OC], f32, tag="psmv")
                    for h in range(H):
                        for r in range(4):
                            nc.tensor.matmul(ps_mv[:, 32 * h:32 * (h + 1)],
                                             xsm_b[:, h, r:r + 1],
                                             wfs_sb[:, h, r, :],
                                             start=(r == 0), stop=(r == 3))
                    nc.scalar.activation(valrow[:, 0:OC], ps_mv[:], Act.Copy)

                    # evict sq rows, reduce over w, ship to the AR input
                    mrow = p_scr.tile([8, M_FREE], f32, tag="mrow")
                    nc.scalar.activation(mrow[:], ps_sq[:], Act.Copy)
                    mred = p_scr.tile([8, OCH], f32, tag="mred")
                    nc.vector.reduce_sum(
                        mred[:],
                        mrow[:].rearrange("p (w o) -> p o w", w=V, o=OCH),
                        axis=mybir.AxisListType.X)
                    nc.sync.dma_start(cc_in[:, 512:768], mred[:])

                # -------- residual GEMMs + stats (interleaved) --------
                # Per (n, v): GEMM -> evict (Pool) -> Σval ones-matmul and
                # Gram accumulation ride in the PE stream right behind.
                with (
                    tc.tile_pool(name="pr", bufs=4, space="PSUM") as p_pr,
                    tc.tile_pool(name="prs", bufs=1, space="PSUM") as p_prs,
                ):
                    ps_rv = p_prs.tile([1, OC], f32, tag="psrv")
                    gps0 = p_prs.tile([128, 128], f32, tag="gram0")
                    gps1 = p_prs.tile([128, 128], f32, tag="gram1")
                    # Σ res values = xsum_natᵀ · rw
                    nc.tensor.matmul(ps_rv[:], xsn_b[:], rw_sb[:],
                                     start=True, stop=True)
                    nc.scalar.activation(valrow[:, OC:2 * OC], ps_rv[:],
                                         Act.Copy)
                    for n in range(NS):
                        for v in range(V):
                            pr = p_pr.tile([128, OC], f32, tag="pres")
                            nc.tensor.matmul(pr[:], x_nat[:, n, :, v],
                                             rw_sb[:], start=True, stop=True)
                            # GPSIMD can't read PSUM on HW; split evictions
                            # between Act and DVE
                            if v % 2 == 0:
                                nc.scalar.activation(res_fin[:, n, v, :],
                                                     pr[:], Act.Copy)
                            else:
                                nc.vector.tensor_copy(res_fin[:, n, v, :],
                                                      pr[:])
                            st = (n == 0 and v == 0)
                            sp = (n == NS - 1 and v == V - 1)
                            for c, gps in ((0, gps0), (1, gps1)):
                                sl = res_fin[:, n, v, 128 * c:128 * (c + 1)]
                                nc.tensor.matmul(gps[:], sl, sl,
                                                 start=st, stop=sp)
                    # Σ res² = diag(Gram)
                    for c, gps in ((0, gps0), (1, gps1)):
                        dsb = p_scr.tile([128, 128], f32, tag="diag")
                        nc.vector.tensor_mul(dsb[:], gps[:], id_sb[:])
                        ps_d = p_prs.tile([1, 128], f32, tag="psd")
                        nc.tensor.matmul(ps_d[:], ones_f[:], dsb[:],
                                         start=True, stop=True)
                        nc.scalar.activation(
                            resrow[:, 128 * c:128 * (c + 1)],
                            ps_d[:], Act.Copy)

            # ---------------- AllReduce ----------
            # AR payload: [mval 256 | rval 256 | msq 256 | rsq 256]
            # (mred -> [512:768] shipped above; spread DMAs across engines)
            nc.scalar.dma_start(cc_in[:, 0:512], valrow[:])
            nc.gpsimd.dma_start(cc_in[:, 768:1024], resrow[:])
            nc.gpsimd.collective_compute(
                "AllReduce", Alu.add,
                replica_groups=[list(range(NCORES))],
                ins=[cc_in[:]], outs=[cc_out[:]])
            # load reduced stats partition-spread: statg32[o', kind, h]
            statg = p_small.tile([32, 4, H], f32, tag="statg")
            nc.sync.dma_start(
                statg[:],
                cc_out[:].rearrange("one (k h o) -> (one o) k h",
                                    k=4, h=H, o=OCH))

            # ---------------- transposed (channel-major) values ---------
            with (
                tc.tile_pool(name="cm", bufs=1) as p_cm,
                tc.tile_pool(name="fo", bufs=2) as p_fo,
                tc.tile_pool(name="pc", bufs=1, space="PSUM") as p_pc,
            ):
                # main_cm/res_cm: [p=(tm,o'), n, w, h, ts]
                main_cm = p_cm.tile([128, NS, V, H, 32], bf16, tag="mcm")
                res_cm = p_cm.tile([128, NS, V, H, 32], bf16, tag="rcm")
                for h in range(H):
                    nc.vector.transpose(main_cm[:, :, :, h, :],
                                        fin[:, :, :, h, :])
                    nc.vector.transpose(
                        res_cm[:, :, :, h, :],
                        res_fin[:, :, :, :].rearrange(
                            "p n w (h o) -> p n w h o", h=H, o=OCH)[:, :, :, h, :])

                # ---------------- coefficients ----------------
                # statg [32, kind, h]: kinds (mval, rval, msq, rsq).
                # Everything on 32 partitions (one per o').
                coef32 = p_small.tile([32, 3, H], f32, tag="coef32")
                AB_v = coef32[:, 0:2, :]
                E_v = coef32[:, 2, :]
                mu = p_small.tile([32, 2, H], f32, tag="cmu")
                mu2 = p_small.tile([32, 2, H], f32, tag="cmu2")
                inv = 1.0 / float(NTOT)

                nc.vector.tensor_scalar_mul(mu[:], statg[:, 0:2, :], inv)
                nc.vector.tensor_mul(mu2[:], mu[:], mu[:])
                # var = sq/N - mu^2
                nc.vector.scalar_tensor_tensor(
                    AB_v, statg[:, 2:4, :], inv, mu2[:],
                    Alu.mult, Alu.subtract)
                # sd = sqrt(var + eps); A,B = gamma / sd
                nc.scalar.activation(AB_v, AB_v, Act.Sqrt, bias=eps_ap[:])
                nc.vector.reciprocal(AB_v, AB_v)
                nc.vector.tensor_mul(AB_v, AB_v, g32_sb[:])
                # E = (b1+b2) - A*mu_m - B*mu_r
                nc.vector.tensor_mul(mu2[:], AB_v, mu[:])
                nc.vector.tensor_sub(E_v, b12_sb[:], mu2[:, 0, :])
                nc.vector.tensor_sub(E_v, E_v, mu2[:, 1, :])

                # broadcast per-o' coef rows to all 128 partitions:
                # cb[p, (c,h)] = coef32[p % 32, c, h]
                cb_ps = p_pc.tile([128, 3 * H], f32, tag="cbps")
                nc.tensor.matmul(cb_ps[:], sel_sb[:],
                                 coef32[:].rearrange("o c h -> o (c h)"),
                                 start=True, stop=True)
                coef = p_small.tile([128, 3 * H], f32, tag="coef")
                nc.vector.tensor_copy(coef[:], cb_ps[:])

                # ---------------- combine + relu + out DMA ----------------
                # head-pair pipeline; each (head-pair, tm) block ships to
                # its own DRAM param with (ts,w)=3200B contiguous runs.
                # Engine assignment keeps SP/Pool even and gives Act the
                # tail DMAs (after its relus are done).
                dma_q = [nc.sync, nc.gpsimd, nc.sync, nc.gpsimd,
                         nc.sync, nc.gpsimd, nc.sync, nc.gpsimd,
                         nc.sync, nc.gpsimd, nc.scalar, nc.scalar,
                         nc.sync, nc.gpsimd, nc.scalar, nc.scalar]
                for h in range(H):
                    A_h = coef[:, h:h + 1]
                    B_h = coef[:, H + h:H + h + 1]
                    E_h = coef[:, 2 * H + h:2 * H + h + 1]
                    t1 = p_fo.tile([128, NS, V, 32], bf16, tag="t1")
                    nc.vector.tensor_scalar(t1[:], res_cm[:, :, :, h, :],
                                            B_h, E_h, Alu.mult, Alu.add)
                    t2b = p_fo.tile([128, NS, V, 32], bf16, tag="t2")
                    nc.vector.scalar_tensor_tensor(
                        t2b[:], main_cm[:, :, :, h, :], A_h, t1[:],
                        Alu.mult, Alu.add)
                    if h % 2 == 0:
                        fo2 = p_fo.tile([128, 2, NS, 32, V], bf16, tag="fo")
                    nc.scalar.activation(
                        fo2[:, h % 2].rearrange("p n ts w -> p n w ts"),
                        t2b[:], Act.Relu)
                    if h % 2 == 1:
                        hp = h // 2
                        for tm in range(4):
                            dma_q[4 * hp + tm].dma_start(
                                outs[4 * hp + tm][:].rearrange(
                                    "g n o ts w -> o (g n) (ts w)"),
                                fo2[32 * tm:32 * (tm + 1)].rearrange(
                                    "p g n ts w -> p (g n) (ts w)"))

    _split_excess_sync(nc)
    return nc


def _make_runner(nc):
    """Build a cached PJRT executor (same lowering path run_bass_kernel_spmd
    uses under axon, but the jit closure is built once so warm calls skip
    re-trace/re-lower)."""
    import jax
    import jax.numpy as jnp
    from jax.sharding import Mesh, PartitionSpec
    from jax.experimental.shard_map import shard_map
    from concourse import bass2jax
    from concourse import mybir

    import jax.numpy as jnp

    bass2jax.install_neuronx_cc_hook()
    partition_name = (nc.partition_id_tensor.name
                      if nc.partition_id_tensor else None)
    # per-core (sharded) vs replicated inputs
    sharded_in = {"xT", "xnat"}
    in_names, out_names, out_avals, zero_outs = [], [], [], []
    for alloc in nc.m.functions[0].allocations:
        if not isinstance(alloc, mybir.MemoryLocationSet):
            continue
        name = alloc.memorylocations[0].name
        if alloc.kind == "ExternalInput":
            if name != partition_name:
                in_names.append(name)
        elif alloc.kind == "ExternalOutput":
            shape = tuple(alloc.tensor_shape)
            dtype = mybir.dt.np(alloc.dtype)
            out_names.append(name)
            out_avals.append(jax.core.ShapedArray(shape, dtype))
            zero_outs.append(np.zeros((NCORES * shape[0], *shape[1:]), dtype))
    n_params = len(in_names)
    all_names = list(in_names) + list(out_names)
    if partition_name is not None:
        all_names.append(partition_name)
    donate = tuple(range(n_params, n_params + len(out_names)))

    def _body(*args):
        operands = list(args)
        if partition_name is not None:
            operands.append(bass2jax.partition_id_tensor())
        return tuple(bass2jax._bass_exec_p.bind(
            *operands,
            out_avals=tuple(out_avals),
            in_names=tuple(all_names),
            out_names=tuple(out_names),
            lowering_input_output_aliases=(),
            sim_require_finite=True,
            sim_require_nnan=True,
            nc=nc,
        ))

    devices = jax.devices()[:NCORES]
    mesh = Mesh(np.asarray(devices), ("core",))
    in_specs = tuple(
        PartitionSpec("core") if nm in sharded_in else PartitionSpec()
        for nm in in_names) + (PartitionSpec("core"),) * len(out_names)
    sharded = jax.jit(
        shard_map(_body, mesh=mesh, in_specs=in_specs,
                  out_specs=(PartitionSpec("core"),) * len(out_names),
                  check_rep=False),
        donate_argnums=donate, keep_unused=True)

    def run(in_maps):
        args = []
        for nm in in_names:
            if nm in sharded_in:
                args.append(np.concatenate(
                    [np.asarray(in_maps[c][nm]) for c in range(NCORES)],
                    axis=0))
            else:
                args.append(np.asarray(in_maps[0][nm]))
        out_arrs = sharded(*args, *zero_outs)
        results = []
        for c in range(NCORES):
            results.append({
                nm: np.asarray(out_arrs[i]).reshape(
                    NCORES, *out_avals[i].shape)[c]
                for i, nm in enumerate(out_names)
            })
        return results

    return run


def kernel(**inputs):
    import sys
    if "/opt/trn_rl_repo" not in sys.path:
        sys.path.insert(0, "/opt/trn_rl_repo")
    from concourse.bass_utils import run_bass_kernel_spmd

    (xT, x_nat, Wf_dev, Wfsum, rwT, gb, sel, ident, eyerep, gb32,
     bet12) = _host_prep(inputs)

    if "nc" not in _CACHED:
        _CACHED["nc"] = _build_bass()
    nc = _CACHED["nc"]

    in_maps = []
    for c in range(NCORES):
        in_maps.append({
            "xT": xT[c],
            "xnat": x_nat[c],
            "wf": Wf_dev,
            "wfs": Wfsum,
            "rwT": rwT,
            "sel": sel,
            "ident": ident,
            "eyerep": eyerep,
            "gb32": gb32,
            "bet12": bet12,
        })
    if "runner" in _CACHED:
        results = _CACHED["runner"](in_maps)
    else:
        # first call goes through the standard entry point (compiles the
        # NEFF); subsequent calls reuse a cached jit executor
        res = run_bass_kernel_spmd(nc, in_maps,
                                   core_ids=list(range(NCORES)))
        results = res.results
        try:
            _CACHED["runner"] = _make_runner(nc)
        except Exception:
            pass
    full = np.empty((N, OC, T, V), np.float32)
    for c in range(NCORES):
        rc = results[c]
        for hp in range(H // 2):
            for tm in range(4):
                blk = rc[f"out_{hp}_{tm}"]        # [2, NS, 32, 32, V] bf16
                for g in range(2):
                    full[c * NS:(c + 1) * NS,
                         32 * (2 * hp + g):32 * (2 * hp + g + 1),
                         32 * tm:32 * (tm + 1), :] = \
                        blk[g].astype(np.float32)
    return full
